# revision 6
# baseline (speedup 1.0000x reference)
"""Trainium2 Bass kernel for nn_Alpha2Assoc: 3-layer alpha compositing.

Hybrid ACT/DVE architecture over disjoint pixel sets:

A-path (rows 0..R_A): partition-major [(b,d)=128, pix]. Per layer:
  u = Ln(1-a) on ScalarE, exclusive cumsum over d via TensorE matmul
  with a block-triangular 0/1 stationary, vis = Exp(PSUM) on ScalarE.
  occ/a/out muls on VectorE (bf16 TS 4x / TT 2x modes).
  ScalarE-bound: 6 passes at ~1 elem/cyc.

B-path (rows R_A..64): pixel-major [pix=128, d-segments of 32 along free
  dim]. Exclusive cumprod via VectorE tensor_tensor_scan:
  state = max(ta[t-1]*state, bnd[t]), bnd = 1 at segment starts
  (exact reset since all products <= 1), ta read through a 1-shifted
  view of a leading-zero-column buffer. B-path groups are issued
  breadth-first across j-tiles and paced to drain before the A-path
  tail rounds.

Measured rates (HW): ACT 1 elem/cyc @1.2GHz; DVE TT 2x / TS 4x bf16;
scan ~2.1 cyc/elem. scalar_tensor_tensor is 1x-only and GPSIMD
elementwise is ~2.5x slower + contends for the DVE SBUF port - both
tried and rejected. The split R_B/64 balances ScalarE (A) against
VectorE (A muls + all of B).
"""

import numpy as np
import ml_dtypes

import concourse.bass as bass
import concourse.tile as tile
from concourse import bacc, mybir
from concourse._compat import with_exitstack
from concourse.bass_utils import run_bass_kernel_spmd

BF16 = ml_dtypes.bfloat16

# --- Pin Ln/Exp to the one table set containing both -------------------
_orig_get_activation_tables = bacc.get_activation_tables


def _pinned_get_activation_tables(arch):
    tables = _orig_get_activation_tables(arch)
    keep = {mybir.ActivationFunctionType.Ln, mybir.ActivationFunctionType.Exp}
    return {
        name: (fns if name == "natural_log_exp_and_others" else fns - keep)
        for name, fns in tables.items()
    }


bacc.get_activation_tables = _pinned_get_activation_tables

B, D, H, W = 4, 32, 512, 512
N_CORES = 8
H_SH = H // N_CORES              # 64 rows per core
P = B * D                        # 128 partitions
SEG = D                          # cumprod segment length in B layout

R_B = 13                         # rows on the B (scan) path, of 64
R_A = H_SH - R_B
N_A = R_A * W
N_B = R_B * W

TILE_A = 3584
_lead = [1024, 1024, 2048]
_tail = [2048, 1024]
_rest = N_A - sum(_lead) - sum(_tail)
A_SIZES = _lead + [TILE_A] * (_rest // TILE_A)
if _rest % TILE_A:
    A_SIZES.append(_rest % TILE_A)
A_SIZES += _tail
A_OFFS = [sum(A_SIZES[:k]) for k in range(len(A_SIZES))]
assert sum(A_SIZES) == N_A

NB_TILES = 3
_bt = (N_B // NB_TILES) // SEG * SEG
B_SIZES = [_bt] * (NB_TILES - 1) + [N_B - _bt * (NB_TILES - 1)]
B_OFFS = [sum(B_SIZES[:k]) for k in range(NB_TILES)]
B_TILE = max(B_SIZES)
MM_CHUNK = 512
PS_N = 2048

F32 = mybir.dt.float32
BF = mybir.dt.bfloat16
AF = mybir.ActivationFunctionType
OP = mybir.AluOpType

_COMPILED = {}


def _tri_matrix() -> np.ndarray:
    k = np.arange(P)
    m = np.arange(P)
    same_b = (k[:, None] // D) == (m[None, :] // D)
    lower = (k[:, None] % D) < (m[None, :] % D)
    return (same_b & lower).astype(np.float32)


@with_exitstack
def _alpha_kernel(ctx, tc, outA_aps, outB_aps, inA_ap, inB_ap, tri_ap, bnd_ap):
    nc = tc.nc
    const_pool = ctx.enter_context(tc.tile_pool(name="const", bufs=1))
    # A-path pools (bf16, TILE_A wide)
    a_pool = ctx.enter_context(tc.tile_pool(name="a", bufs=2))
    u_pool = ctx.enter_context(tc.tile_pool(name="u", bufs=2))
    vis_pool = ctx.enter_context(tc.tile_pool(name="vis", bufs=4))
    occ_pool = ctx.enter_context(tc.tile_pool(name="occ", bufs=2))
    s_pool = ctx.enter_context(tc.tile_pool(name="s", bufs=3))
    o_pool = ctx.enter_context(tc.tile_pool(name="o", bufs=2))
    psum_pool = ctx.enter_context(tc.tile_pool(name="ps", bufs=2, space="PSUM"))
    # B-path pools
    ab_pool = ctx.enter_context(tc.tile_pool(name="ab", bufs=3))
    ta_pool = ctx.enter_context(tc.tile_pool(name="ta", bufs=4))
    visb_pool = ctx.enter_context(tc.tile_pool(name="visb", bufs=4))
    nvb_pool = ctx.enter_context(tc.tile_pool(name="nvb", bufs=4))
    sb_pool = ctx.enter_context(tc.tile_pool(name="sb", bufs=4))
    ob_pool = ctx.enter_context(tc.tile_pool(name="ob", bufs=2))

    # tiny dummy activation: forces the Ln/Exp ACT table load (~2.7us)
    # to overlap the first input DMAs instead of stalling the first Ln
    warm = const_pool.tile([P, 8], BF)
    nc.vector.memset(warm[:], 0.0)
    nc.scalar.activation(warm[:], warm[:], AF.Ln, bias=1.0, scale=-1.0)

    tri = const_pool.tile([P, P], BF)
    nc.sync.dma_start(tri[:], tri_ap[:, :])
    bnd = const_pool.tile([P, B_TILE], BF)
    nc.sync.dma_start(bnd[:], bnd_ap[:, 0:B_TILE])

    # pre-zero the leading column of every ta buffer (shifted-view reset
    # reads it; writes only ever touch cols 1..N so it stays zero)
    for _ in range(4):
        t = ta_pool.tile([P, B_TILE + 1], BF, tag="ta")
        nc.vector.memset(t[:, 0:1], 0.0)

    def cumsum_mm(u, n):
        """Issue matmuls for one layer-tile; return psum tiles for exp."""
        pss = []
        for h in range((n + PS_N - 1) // PS_N):
            w = min(PS_N, n - h * PS_N)
            ps = psum_pool.tile([P, PS_N], F32, tag="ps")
            for j in range((w + MM_CHUNK - 1) // MM_CHUNK):
                mc = min(MM_CHUNK, w - j * MM_CHUNK)
                nc.tensor.matmul(
                    ps[:, bass.ds(j * MM_CHUNK, mc)],
                    tri[:],
                    u[:, bass.ds(h * PS_N + j * MM_CHUNK, mc)],
                    start=True,
                    stop=True,
                )
            pss.append((ps, w))
        return pss

    def exp_drain(pss, vis):
        off = 0
        for ps, w in pss:
            nc.scalar.activation(
                vis[:, bass.ds(off, w)], ps[:, bass.ds(0, w)], AF.Exp
            )
            off += w

    # ---------------- A-path stages (software-pipelined over tiles) ----
    st = {}

    def st_a_ln(i):
        n = A_SIZES[i]
        a1 = a_pool.tile([P, n], BF, tag="a")
        nsp = 4 if n >= 2048 else 2
        hh = n // nsp
        for k in range(nsp):
            w = hh if k < nsp - 1 else n - hh * (nsp - 1)
            nc.sync.dma_start(a1[:, bass.ds(k * hh, w)],
                              inA_ap[:, bass.ds(A_OFFS[i] + k * hh, w)])
        u1 = u_pool.tile([P, n], BF, tag="u")
        nc.scalar.activation(u1[:], a1[:], AF.Ln, bias=1.0, scale=-1.0)
        st[i] = {"a1": a1, "ps1": cumsum_mm(u1, n)}

    def st_a_exp(i):
        n = A_SIZES[i]
        sl = bass.ds(A_OFFS[i], n)
        vis1 = vis_pool.tile([P, n], BF, tag="vis")
        exp_drain(st[i].pop("ps1"), vis1)
        nc.sync.dma_start(outA_aps[0][:, sl], vis1[:])
        st[i]["vis1"] = vis1

    def st_b_pre(i):
        n = A_SIZES[i]
        a1, vis1 = st[i].pop("a1"), st[i].pop("vis1")
        occ1 = occ_pool.tile([P, n], BF, tag="occ")
        nc.vector.tensor_scalar(occ1[:], vis1[:], -1.0, 1.0, OP.mult, OP.add)
        a2 = s_pool.tile([P, n], BF, tag="s")
        nc.vector.tensor_tensor(a2[:], a1[:], occ1[:], OP.mult)
        u2 = u_pool.tile([P, n], BF, tag="u")
        nc.scalar.activation(u2[:], a2[:], AF.Ln, bias=1.0, scale=-1.0)
        st[i].update({"a2": a2, "occ1": occ1, "ps2": cumsum_mm(u2, n)})

    def st_b_exp(i):
        n = A_SIZES[i]
        sl = bass.ds(A_OFFS[i], n)
        vis2 = vis_pool.tile([P, n], BF, tag="vis")
        exp_drain(st[i].pop("ps2"), vis2)
        o2 = o_pool.tile([P, n], BF, tag="o")
        nc.vector.tensor_tensor(o2[:], vis2[:], st[i].pop("occ1")[:], OP.mult)
        nc.sync.dma_start(outA_aps[1][:, sl], o2[:])
        st[i]["vis2"] = vis2

    def st_c_pre(i):
        n = A_SIZES[i]
        a2, vis2 = st[i].pop("a2"), st[i].pop("vis2")
        occ2 = occ_pool.tile([P, n], BF, tag="occ")
        nc.vector.tensor_scalar(occ2[:], vis2[:], -1.0, 1.0, OP.mult, OP.add)
        a3 = s_pool.tile([P, n], BF, tag="s")
        nc.vector.tensor_tensor(a3[:], a2[:], occ2[:], OP.mult)
        u3 = u_pool.tile([P, n], BF, tag="u")
        nc.scalar.activation(u3[:], a3[:], AF.Ln, bias=1.0, scale=-1.0)
        st[i].update({"occ2": occ2, "ps3": cumsum_mm(u3, n)})

    def st_c_exp(i):
        n = A_SIZES[i]
        sl = bass.ds(A_OFFS[i], n)
        vis3 = vis_pool.tile([P, n], BF, tag="vis")
        exp_drain(st[i].pop("ps3"), vis3)
        o3 = o_pool.tile([P, n], BF, tag="o")
        nc.vector.tensor_tensor(o3[:], vis3[:], st[i].pop("occ2")[:], OP.mult)
        nc.sync.dma_start(outA_aps[2][:, sl], o3[:])
        del st[i]

    # ---------------- B-path op groups -------------------------------
    # Breadth-first waves across j-tiles keep the in-order DVE queue from
    # head-of-line blocking on same-tile dependencies.
    stbs = [{} for _ in range(len(B_SIZES))]

    def b_groups():
        def g_dma(j):
            n = B_SIZES[j]
            ab = ab_pool.tile([P, n], BF, tag="ab")
            h = n // 2
            nc.sync.dma_start(ab[:, 0:h], inB_ap[:, bass.ds(B_OFFS[j], h)])
            nc.sync.dma_start(ab[:, h:n], inB_ap[:, bass.ds(B_OFFS[j] + h, n - h)])
            stbs[j]["ab"] = ab

        def g_ta1(j):
            n, stb = B_SIZES[j], stbs[j]
            ta = ta_pool.tile([P, B_TILE + 1], BF, tag="ta")
            nc.vector.tensor_scalar(ta[:, 1:n + 1], stb["ab"][:], -1.0, 1.0,
                                    OP.mult, OP.add)
            stb["ta"] = ta

        def g_scan1(j):
            n, stb = B_SIZES[j], stbs[j]
            vis1 = visb_pool.tile([P, n], BF, tag="visb")
            nc.vector.tensor_tensor_scan(vis1[:], stb.pop("ta")[:, 0:n],
                                         bnd[:, 0:n], 1.0, OP.mult, OP.max)
            nc.sync.dma_start(outB_aps[0][:, bass.ds(B_OFFS[j], n)], vis1[:])
            stb["vis1"] = vis1

        def g_l2pre(j):
            n, stb = B_SIZES[j], stbs[j]
            nv1 = nvb_pool.tile([P, n], BF, tag="nvb")
            nc.vector.tensor_scalar(nv1[:], stb.pop("vis1")[:], -1.0, 1.0,
                                    OP.mult, OP.add)
            a2 = sb_pool.tile([P, n], BF, tag="sb")
            nc.vector.tensor_tensor(a2[:], stb.pop("ab")[:], nv1[:], OP.mult)
            ta = ta_pool.tile([P, B_TILE + 1], BF, tag="ta")
            nc.vector.tensor_scalar(ta[:, 1:n + 1], a2[:], -1.0, 1.0,
                                    OP.mult, OP.add)
            stb.update(nv1=nv1, a2=a2, ta=ta)

        def g_scan2(j):
            n, stb = B_SIZES[j], stbs[j]
            vis2 = visb_pool.tile([P, n], BF, tag="visb")
            nc.vector.tensor_tensor_scan(vis2[:], stb.pop("ta")[:, 0:n],
                                         bnd[:, 0:n], 1.0, OP.mult, OP.max)
            o2 = ob_pool.tile([P, n], BF, tag="ob")
            nc.vector.tensor_tensor(o2[:], vis2[:], stb.pop("nv1")[:], OP.mult)
            nc.sync.dma_start(outB_aps[1][:, bass.ds(B_OFFS[j], n)], o2[:])
            nv2 = nvb_pool.tile([P, n], BF, tag="nvb")
            nc.vector.tensor_scalar(nv2[:], vis2[:], -1.0, 1.0,
                                    OP.mult, OP.add)
            stb["nv2"] = nv2

        def g_l3pre(j):
            n, stb = B_SIZES[j], stbs[j]
            a3 = sb_pool.tile([P, n], BF, tag="sb")
            nc.vector.tensor_tensor(a3[:], stb.pop("a2")[:], stb["nv2"][:],
                                    OP.mult)
            ta = ta_pool.tile([P, B_TILE + 1], BF, tag="ta")
            nc.vector.tensor_scalar(ta[:, 1:n + 1], a3[:], -1.0, 1.0,
                                    OP.mult, OP.add)
            stb["ta"] = ta

        def g_scan3(j):
            n, stb = B_SIZES[j], stbs[j]
            vis3 = visb_pool.tile([P, n], BF, tag="visb")
            nc.vector.tensor_tensor_scan(vis3[:], stb.pop("ta")[:, 0:n],
                                         bnd[:, 0:n], 1.0, OP.mult, OP.max)
            o3 = ob_pool.tile([P, n], BF, tag="ob")
            nc.vector.tensor_tensor(o3[:], vis3[:], stb.pop("nv2")[:], OP.mult)
            nc.sync.dma_start(outB_aps[2][:, bass.ds(B_OFFS[j], n)], o3[:])

        js = range(len(B_SIZES))
        for j in js:
            yield (lambda j=j: g_dma(j))
        for g in (g_ta1, g_scan1, g_l2pre, g_scan2, g_l3pre, g_scan3):
            for j in js:
                yield (lambda j=j, g=g: g(j))

    bq = b_groups()
    NT = len(A_SIZES)
    n_groups = 7 * len(B_SIZES)
    # pace B to finish by ~round NT (before the 2 drain rounds)
    n_slots = 3 * NT
    popped = [0, 0]  # groups popped, slots seen

    def pop_b():
        popped[1] += 1
        want = min((popped[1] * n_groups + n_slots - 1) // n_slots, n_groups)
        while popped[0] < want:
            try:
                next(bq)()
            except StopIteration:
                return
            popped[0] += 1

    # ---------------- schedule -----------------------------------------
    for r in range(NT + 2):
        if r < NT:
            st_a_ln(r)
        if 1 <= r <= NT:
            st_b_pre(r - 1)
        pop_b()
        if r < NT:
            st_a_exp(r)
        if 2 <= r:
            st_c_pre(r - 2)
        pop_b()
        if 1 <= r <= NT:
            st_b_exp(r - 1)
        if 2 <= r:
            st_c_exp(r - 2)
        pop_b()
    for _ in range(n_groups):
        pop_b()


def _build():
    nc = bacc.Bacc("TRN2", target_bir_lowering=False, debug=False,
                   num_devices=N_CORES)
    inA_ap = nc.dram_tensor("alphaA", [P, N_A], BF, kind="ExternalInput").ap()
    inB_ap = nc.dram_tensor("alphaB", [P, N_B], BF, kind="ExternalInput").ap()
    tri_ap = nc.dram_tensor("tri", [P, P], BF, kind="ExternalInput").ap()
    bnd_ap = nc.dram_tensor("bnd", [P, B_TILE], BF,
                            kind="ExternalInput").ap()
    outA_aps = [
        nc.dram_tensor(f"outA{c}", [P, N_A], BF, kind="ExternalOutput").ap()
        for c in range(3)
    ]
    outB_aps = [
        nc.dram_tensor(f"outB{c}", [P, N_B], BF, kind="ExternalOutput").ap()
        for c in range(3)
    ]
    with tile.TileContext(nc) as tc:
        _alpha_kernel(tc, outA_aps, outB_aps, inA_ap, inB_ap, tri_ap, bnd_ap)
    nc.compile()
    return nc


def _get_nc():
    if "nc" not in _COMPILED:
        _COMPILED["nc"] = _build()
    return _COMPILED["nc"]


def _run(alpha_imgs: np.ndarray, trace: bool = False):
    nc = _get_nc()
    tri = _tri_matrix().astype(BF16)
    bndv = np.zeros((P, B_TILE), dtype=np.float32)
    bndv[:, 0::SEG] = 1.0
    bndv = bndv.astype(BF16)
    # clamp below 1: bf16 rounding can hit 1.0 exactly, and the A-path's
    # Ln(1-a) -> -inf would poison the matmul (0 * -inf = NaN)
    a = np.minimum(np.asarray(alpha_imgs)[:, :, 0], np.float32(1 - 2**-8))
    a = a.astype(BF16)  # [B, D, H, W] bf16
    in_maps = []
    for c in range(N_CORES):
        h0 = c * H_SH
        shA = np.ascontiguousarray(a[:, :, h0:h0 + R_A, :]).reshape(P, N_A)
        # B rows -> pixel-major: [B, D, R_B, W] -> [B, R_B, W, D] -> [128, -1]
        shB = np.ascontiguousarray(
            a[:, :, h0 + R_A:h0 + H_SH, :].transpose(0, 2, 3, 1)
        ).reshape(P, N_B)
        in_maps.append({"alphaA": shA, "alphaB": shB, "tri": tri, "bnd": bndv})
    res = None
    backoffs = [3.0, 10.0, 30.0, 60.0]
    for attempt in range(len(backoffs) + 1):
        try:
            res = run_bass_kernel_spmd(
                nc, in_maps, core_ids=list(range(N_CORES)), trace=trace
            )
            break
        except Exception:
            if attempt == len(backoffs):
                raise
            import time

            time.sleep(backoffs[attempt])
    out = np.empty((B, D, 3, H, W), dtype=np.float32)
    for c in range(N_CORES):
        r = res.results[c]
        h0 = c * H_SH
        oA = np.stack([r["outA0"], r["outA1"], r["outA2"]], axis=1)
        oA = oA.astype(np.float32).reshape(B, D, 3, R_A, W)
        out[:, :, :, h0:h0 + R_A, :] = oA
        # [128, 3, N_B] pixel-major -> [B, D, 3, R_B, W]
        oB = np.stack([r["outB0"], r["outB1"], r["outB2"]], axis=1)
        oB = oB.astype(np.float32)
        ob = oB.reshape(P, 3, N_B // SEG, SEG)           # [part, l, px/part, d]
        ob = ob.transpose(1, 0, 2, 3).reshape(3, B, R_B, W, SEG)
        out[:, :, :, h0 + R_A:h0 + H_SH, :] = ob.transpose(1, 4, 0, 2, 3)
    return out, res


def kernel(alpha_imgs: np.ndarray) -> np.ndarray:
    out, _ = _run(alpha_imgs, trace=False)
    return out
